# revision 15
# baseline (speedup 1.0000x reference)
"""GAT (2-layer, DGL-style GATConv) on 8 Trainium2 NeuronCores.

Design (v2): dst-node partition (graph parallel), per-edge pull via windowed
int16 dma_gather spread over 4 SWDGE queues.

Key mechanisms (all HW-probed this session):
  - dma_gather desc-gen runs on the Q7 pair selected by queue_num (cpu_id/2 ==
    queue_num); round-robin over 4 queues overlaps 4 calls -> ~2.4ns/row vs
    ~9ns/row for the old per-column indirect_dma_start path. Needs
    num_swdge_queues=4 and a bigger descriptor carveout
    (dynamic_dma_scratch_size=131072) so a queue's next call doesn't wait for
    the previous call's ring to drain.
  - dma_gather takes int16 idxs (wrapped [16, n/16], replicated x8 across
    partition groups), gathers rows of elem_size (256B multiple) with ordinal
    placement j -> [j%128, j//128], so each call fills an 8-column block of a
    [128, K] slot grid. Table split into 4 windows of <=32768 rows (int16
    range); per-call base = window start. Trailing-negative idxs are trimmed
    by the Q7 BEFORE desc-gen (per-core work reduction lever, unused so far).
  - Rotation trick: fold attn_l into an invertible per-head basis change R
    (col 0 of R_h = al_h), so el IS column h*d of the projected features.
    Table rows carry just the rotated features (128 bf16 = 256B exactly, no
    el/hi/lo columns). Layer-0 output is un-rotated on device by one PE
    matmul (lhsT=Rinv) fused before the elu; layer-1 output un-rotated on the
    host (logits = out @ R1inv).

Host layout: degree-snake core assignment; within each core, nodes lexsorted
by their per-window in-degree profile (3 fixpoint iterations, windows =
32768-aligned table rows) and chunked into 128-row dst tiles. Slot grid per
tile: columns grouped by window, each dst's window-w slots padded to the
tile's per-window max (shared across cores) with that window's dummy row
(features 0, el cols -1e9 -> exp contributes 0). Measured padding ~1.86x.

Perf notes / dead ends (HW-probed, do not retry): multi-column offset APs for
indirect_dma_start truly corrupt data (element-subsampled, not a
permutation); dma_gather on ONE queue costs 8.6us/1024 idxs (scalar-ish Q7
loop), the cost model's 0.34ns/desc is wrong for it; DMA transfer side
aggregates 64 random 256B rows/engine into ~1us bursts (never the
bottleneck); dma_scatter_add ~10us/call; segmented overlapped AllGathers
lose; int16 range is signed (no uint16 windows).
"""
import sys

sys.path.insert(0, "/opt/trn_rl_repo")

from contextlib import ExitStack

import os

import numpy as np

P = 128
NC = 8
SLOPE = 0.2
WIN = 32768
NWIN = 4
WBUFS = int(os.environ.get("WBUFS", "3"))
GBUFS = int(os.environ.get("GBUFS", "4"))
SCRATCH = int(os.environ.get("SCRATCH", "131072"))
ACT_E = bool(int(os.environ.get("ACT_E", "1")))
ACT_ELU = bool(int(os.environ.get("ACT_ELU", "1")))


def _host_shard(src, dst, n_nodes):
    """Core assignment + windowed slot layout.

    Returns per-core node lists, shared Kw[t, w], wrapped idx arrays, and
    layout metadata."""
    deg = np.bincount(dst, minlength=n_nodes)
    order = np.argsort(-deg, kind="stable")
    i = np.arange(n_nodes)
    r, j = i // NC, i % NC
    core_of_order = np.where(r % 2 == 0, j, NC - 1 - j)
    core = np.empty(n_nodes, np.int64)
    core[order] = core_of_order
    nsh = n_nodes // NC
    vsh = nsh + 1
    ntt = (nsh + P - 1) // P

    # fixpoint: within-core profile sort <-> 32768-aligned window membership
    pos = np.empty(n_nodes, np.int64)
    for c in range(NC):
        nodes = order[core_of_order == c]
        pos[nodes] = np.arange(nsh)
    for _ in range(3):
        newid = core * vsh + pos
        win_node = newid // WIN
        kw = np.zeros((n_nodes, NWIN), np.int64)
        np.add.at(kw, (dst, win_node[src]), 1)
        for c in range(NC):
            nodes = np.where(core == c)[0]
            kk = kw[nodes]
            o = np.lexsort((-kk[:, 3], -kk[:, 2], -kk[:, 1], -kk[:, 0]))
            pos[nodes[o]] = np.arange(len(nodes))
    newid = core * vsh + pos
    win_node = (newid // WIN).astype(np.int64)
    kw = np.zeros((n_nodes, NWIN), np.int64)
    np.add.at(kw, (dst, win_node[src]), 1)

    tile_of = pos // P
    part_of = pos % P
    Kw = np.zeros((ntt, NWIN), np.int64)
    for w in range(NWIN):
        np.maximum.at(Kw, (tile_of, np.full(n_nodes, w)), kw[:, w])
    # guarantee at least 1 column per tile overall (degenerate tiles)
    allz = Kw.sum(1) == 0
    Kw[allz, 0] = 1

    K = Kw.sum(1)                      # cols per tile
    coff = np.zeros((ntt, NWIN), np.int64)   # col offset of window within tile
    coff[:, 1:] = np.cumsum(Kw, axis=1)[:, :-1]
    tile_col0 = np.concatenate([[0], np.cumsum(K)])  # global col offset per tile
    tot_cols = int(tile_col0[-1])
    slots = tot_cols * P

    # dummy row (local row nsh of each core block): pick one inside each
    # window. Dummy el = -100 (not -1e9: bf16 fine either way, but keep
    # lrelu/exp in comfortable range; exp(0.2*-100) ~ 2e-9 ~ 0).
    dummies_g = np.array([c * vsh + nsh for c in range(NC)], np.int64)
    dummy_rel = np.zeros(NWIN, np.int64)
    for w in range(NWIN):
        lo, hi = w * WIN, min((w + 1) * WIN, NC * vsh)
        cand = dummies_g[(dummies_g >= lo) & (dummies_g < hi)]
        assert len(cand) > 0
        dummy_rel[w] = cand[0] - lo

    # per-core idx arrays
    src_n = newid[src]
    ecore = core[dst]
    et = tile_of[dst]
    ep = part_of[dst]
    ew = win_node[src]
    idx_flat_val = (src_n - ew * WIN).astype(np.int64)

    idx_wrapped = np.empty((NC, P, tot_cols * 8), np.int16)
    for c in range(NC):
        m = ecore == c
        t_c, p_c, w_c, v_c = et[m], ep[m], ew[m], idx_flat_val[m]
        o = np.lexsort((p_c, w_c, t_c))
        t_c, p_c, w_c, v_c = t_c[o], p_c[o], w_c[o], v_c[o]
        # rank within (t, w, p)
        key = (t_c * NWIN + w_c) * P + p_c
        first = np.searchsorted(key, key, side="left")
        k_c = np.arange(len(key)) - first
        # flat ordinal within the global column-major slot space
        col = tile_col0[t_c] + coff[t_c, w_c] + k_c
        flat = col * P + p_c
        idx_all = np.empty(slots, np.int16)
        # fill dummies per (t, w) block
        for w in range(NWIN):
            # all slots of window w across tiles
            pass
        # vectorized dummy fill: build per-col window id
        colw = np.empty(tot_cols, np.int64)
        for t in range(ntt):
            for w in range(NWIN):
                colw[tile_col0[t] + coff[t, w]: tile_col0[t] + coff[t, w] + Kw[t, w]] = w
        idx_all[:] = np.repeat(dummy_rel[colw], P).astype(np.int16)
        idx_all[flat] = v_c.astype(np.int16)
        # wrap: ordinal j -> [j%16, j//16], replicate x8
        wrapped = idx_all.reshape(-1, 16).T  # [16, slots/16]
        idx_wrapped[c] = np.tile(wrapped, (8, 1))

    perm_c = []
    for c in range(NC):
        nodes = np.where(core == c)[0]
        pc = np.empty(nsh, np.int64)
        pc[pos[nodes]] = nodes
        perm_c.append(pc)

    return perm_c, Kw, coff, tile_col0, idx_wrapped, nsh, vsh, ntt


def _build_program(n_in, h0, d0, h1, c1, Kw, coff, tile_col0, nsh, vsh, ntt):
    import concourse.bass as bass
    import concourse.mybir as mybir
    from concourse import tile, bacc
    from concourse.masks import make_identity

    f0 = h0 * d0          # 128
    f1 = h1 * c1          # 40
    V = NC * vsh
    AF = mybir.ActivationFunctionType
    OP = mybir.AluOpType
    dt = mybir.dt
    tot_cols = int(tile_col0[-1])
    K = Kw.sum(1)

    nc = bacc.Bacc(num_swdge_queues=4, dynamic_dma_scratch_size=SCRATCH)
    xT = nc.declare_dram_parameter("xT", [n_in, nsh], dt.bfloat16, isOutput=False)
    eidx = nc.declare_dram_parameter("eidx", [P, tot_cols * 8], dt.int16, isOutput=False)
    w0cat = nc.declare_dram_parameter("w0cat", [n_in, f0 + h0], dt.bfloat16, isOutput=False)
    w1cat = nc.declare_dram_parameter("w1cat", [f0, f1 + h1], dt.float32, isOutput=False)
    # colsum of w1cat, replicated across partitions (the elu "-1" fold)
    w1bias = nc.declare_dram_parameter("w1bias", [P, f1 + h1], dt.float32, isOutput=False)
    rinv0 = nc.declare_dram_parameter("rinv0", [f0, f0], dt.float32, isOutput=False)
    out_d = nc.declare_dram_parameter("out", [nsh, c1], dt.float32, isOutput=True)

    tab0_sh = nc.dram_tensor("tab0_sh", [vsh, f0], dt.bfloat16)
    tab0 = nc.dram_tensor("tab0", [V, f0], dt.bfloat16, addr_space="Shared")
    tab1_sh = nc.dram_tensor("tab1_sh", [vsh, f0], dt.bfloat16)
    tab1 = nc.dram_tensor("tab1", [V, f0], dt.bfloat16, addr_space="Shared")

    KCH = n_in // P
    qrr = [0]  # round-robin queue counter

    TGRP = 10  # tiles per idx chunk

    with ExitStack() as ctx:
        er0_sb = ctx.enter_context(nc.sbuf_tensor([P, ntt, h0], dt.float32))
        er1_sb = ctx.enter_context(nc.sbuf_tensor([P, ntt, h1], dt.float32))
        w1_sb = ctx.enter_context(nc.sbuf_tensor([P, f1 + h1], dt.float32))
        w1b_sb = ctx.enter_context(nc.sbuf_tensor([P, f1 + h1], dt.float32))
        rinv0_sb = ctx.enter_context(nc.sbuf_tensor([P, f0], dt.float32))
        ident = ctx.enter_context(nc.sbuf_tensor([P, P], dt.float32))

        # ---------- context 1: preamble + P0 projection ----------
        with tile.TileContext(nc) as tc:
            with (
                tc.tile_pool(name="work", bufs=WBUFS) as wp,
                tc.tile_pool(name="psum", bufs=2, space="PSUM") as psp,
                tc.tile_pool(name="wconst", bufs=1) as wc,
            ):
                w0_sb = wc.tile([P, KCH, f0 + h0], dt.bfloat16)
                nc.sync.dma_start(
                    out=w0_sb[:], in_=w0cat[:].rearrange("(c p) w -> p c w", p=P)
                )
                nc.sync.dma_start(out=w1_sb[:], in_=w1cat[:])
                nc.sync.dma_start(out=w1b_sb[:], in_=w1bias[:])
                nc.sync.dma_start(out=rinv0_sb[:], in_=rinv0[:])
                nc.gpsimd.memset(er0_sb[:], 0.0)
                nc.gpsimd.memset(er1_sb[:], 0.0)
                make_identity(nc, ident[:])

                drow = wp.tile([1, f0], dt.bfloat16, tag="drow")
                nc.gpsimd.memset(drow[:], 0.0)
                for h in range(h0):
                    nc.gpsimd.memset(drow[:, h * d0:h * d0 + 1], -100.0)
                nc.sync.dma_start(out=tab0_sh[nsh:nsh + 1, :], in_=drow[:])
                drow1 = wp.tile([1, f0], dt.bfloat16, tag="drow1")
                nc.gpsimd.memset(drow1[:], 0.0)
                nc.gpsimd.memset(drow1[:, 0:1], -100.0)
                nc.sync.dma_start(out=tab1_sh[nsh:nsh + 1, :], in_=drow1[:])

                for t in range(ntt):
                    nn = min(P, nsh - t * P)
                    ps = psp.tile([P, f0 + h0], dt.float32, tag="proj")
                    xk = wp.tile([P, KCH, P], dt.bfloat16, tag="xk")
                    nc.sync.dma_start(
                        out=xk[:, :, :nn],
                        in_=xT[:, t * P:t * P + nn].rearrange("(c p) n -> p c n", p=P),
                    )
                    for kc in range(KCH):
                        nc.tensor.matmul(
                            ps[:nn, :], lhsT=xk[:, kc, :nn], rhs=w0_sb[:, kc, :],
                            start=(kc == 0), stop=(kc == KCH - 1),
                        )
                    row = wp.tile([P, f0], dt.bfloat16, tag="row")
                    nc.scalar.activation(row[:nn, :], ps[:nn, :f0], AF.Copy)
                    nc.vector.tensor_copy(er0_sb[:nn, t, :], ps[:nn, f0:])
                    nc.sync.dma_start(out=tab0_sh[t * P:t * P + nn, :], in_=row[:nn, :])

        def gather_tile(g, t, tab, ic, icbase):
            for w in range(NWIN):
                kww = int(Kw[t, w])
                co = int(coff[t, w])
                q0 = int(tile_col0[t]) + co
                base = w * WIN
                wrows = min(WIN, V - base)
                for c0 in range(0, kww, 8):
                    ncols = min(8, kww - c0)
                    ni = 128 * ncols
                    nc.gpsimd.dma_gather(
                        g[:, co + c0:co + c0 + ncols, :],
                        tab[base:base + wrows, :],
                        ic[:, (q0 + c0 - icbase) * 8:(q0 + c0 + ncols - icbase) * 8],
                        ni, ni, f0,
                        queue_num=qrr[0] % 4,
                    )
                    qrr[0] += 1

        def edge_phase(wp, gp, ip, tab, hh, dd, er_sb, sink, tagp):
            ff = hh * dd
            # sink(t) is deferred by one tile: each engine queue then sees
            # tile t+1's independent front ops before tile t's chain-dependent
            # sink ops, so the in-order engine queues pipeline across tiles.
            pending = [None]
            for g0 in range(0, ntt, TGRP):
                g1 = min(g0 + TGRP, ntt)
                icbase = int(tile_col0[g0])
                iccols = int(tile_col0[g1]) - icbase
                ic = ip.tile([P, iccols * 8], dt.int16, tag="idx" + tagp)
                nc.sync.dma_start(
                    out=ic[:], in_=eidx[:, icbase * 8:(icbase + iccols) * 8]
                )
                for t in range(g0, g1):
                    edge_tile(wp, gp, tab, hh, dd, ff, er_sb, sink, tagp, t, ic,
                              icbase, pending)
            if pending[0] is not None:
                sink(*pending[0])

        def edge_tile(wp, gp, tab, hh, dd, ff, er_sb, sink, tagp, t, ic, icbase,
                      pending):
                Kt = int(K[t])
                nn = min(P, nsh - t * P)
                g = gp.tile([P, Kt, f0], dt.bfloat16, tag="G" + tagp)
                gather_tile(g, t, tab, ic, icbase)
                # e-score pipeline on the ACT engine, one (lrelu, exp+accum)
                # pair per head: e = exp(lrelu(el + er)), den = sum_k e
                e_bf = wp.tile([P, hh, Kt], dt.bfloat16, tag="eb" + tagp)
                den = wp.tile([P, hh], dt.float32, tag="den" + tagp)
                el_all = g[:].rearrange("p k (h d) -> p h d k", h=hh)
                if ACT_E:
                    for h in range(hh):
                        tmp = wp.tile([P, Kt], dt.float32, tag="tmp" + tagp)
                        nc.scalar.activation(
                            tmp[:], el_all[:, h, 0, :], AF.Prelu,
                            bias=er_sb[:, t, h:h + 1], alpha=SLOPE,
                        )
                        nc.scalar.activation(
                            e_bf[:, h, :], tmp[:], AF.Exp,
                            accum_out=den[:, h:h + 1],
                        )
                else:
                    e_sb = wp.tile([P, hh, Kt], dt.float32, tag="e" + tagp)
                    nc.vector.tensor_tensor(
                        out=e_sb[:], in0=el_all[:, :, 0:1, :].squeeze(2),
                        in1=er_sb[:, t, :].to_broadcast([P, hh, Kt]), op=OP.add,
                    )
                    lk = wp.tile([P, hh, Kt], dt.float32, tag="lk" + tagp)
                    nc.vector.tensor_scalar_mul(lk[:], e_sb[:], SLOPE)
                    nc.vector.tensor_tensor(out=e_sb[:], in0=e_sb[:], in1=lk[:], op=OP.max)
                    nc.scalar.activation(e_bf[:], e_sb[:], AF.Exp)
                    nc.vector.tensor_reduce(den[:], e_bf[:], axis=mybir.AxisListType.X, op=OP.add)
                    nc.vector.tensor_scalar_max(den[:], den[:], 1e-9)
                rec = wp.tile([P, hh], dt.float32, tag="rec" + tagp)
                nc.vector.reciprocal(rec[:], den[:])
                # alpha-weight the gathered rows in place (bf16 x bf16)
                fslice = g[:, :, :ff]
                nc.vector.tensor_tensor(
                    out=fslice.rearrange("p k (h d) -> p k h d", h=hh),
                    in0=fslice.rearrange("p k (h d) -> p k h d", h=hh),
                    in1=e_bf[:].rearrange("p h k -> p k h").to_broadcast([P, Kt, hh, dd]),
                    op=OP.mult,
                )
                orw = wp.tile([P, ff], dt.float32, tag="oraw" + tagp)
                nc.vector.tensor_reduce(
                    orw[:], fslice.rearrange("p k f -> p f k"),
                    axis=mybir.AxisListType.X, op=OP.add,
                )
                if pending[0] is not None:
                    sink(*pending[0])
                pending[0] = (t, nn, orw, rec, hh, dd, wp)

        # ---------- context 2: AllGather0 + E0 + fused P1 ----------
        with tile.TileContext(nc) as tc:
            with (
                tc.tile_pool(name="work", bufs=WBUFS) as wp,
                tc.tile_pool(name="gbuf", bufs=GBUFS) as gp,
                tc.tile_pool(name="ipool0", bufs=2) as ip0,
                tc.tile_pool(name="psum1", bufs=2, space="PSUM") as psp1,
            ):
                nc.gpsimd.collective_compute(
                    "AllGather", mybir.AluOpType.bypass,
                    ins=[tab0_sh[:]], outs=[tab0[:]],
                    replica_groups=[list(range(NC))],
                )

                def sink0(t, nn, orw, rec, hh, dd, wp):
                    # x0B = alpha-normalized aggregate (rotated basis)
                    x0 = wp.tile([P, f0], dt.float32, tag="x0")
                    nc.vector.tensor_tensor(
                        out=x0[:].rearrange("p (h d) -> p h d", h=hh),
                        in0=orw[:].rearrange("p (h d) -> p h d", h=hh),
                        in1=rec[:].to_broadcast([P, hh, dd]),
                        op=OP.mult,
                    )
                    # transpose, un-rotate (hT = Rinv0^T @ x0^T), elu, project.
                    # elu+1 = relu(x) + exp(min(x,0)); the -1 is folded into
                    # w1bias (colsum of w1cat) subtracted after the matmul.
                    xT_ps = psp1.tile([P, P], dt.float32, tag="xT")
                    nc.tensor.transpose(out=xT_ps[:], in_=x0[:], identity=ident[:])
                    x0T = wp.tile([P, P], dt.float32, tag="x0T")
                    nc.scalar.activation(x0T[:], xT_ps[:], AF.Copy)
                    hT_ps = psp1.tile([P, P], dt.float32, tag="hT")
                    nc.tensor.matmul(
                        hT_ps[:, :nn], lhsT=rinv0_sb[:], rhs=x0T[:, :nn],
                        start=True, stop=True,
                    )
                    mneg = wp.tile([P, P], dt.float32, tag="mneg")
                    relu = wp.tile([P, P], dt.float32, tag="relu")
                    if ACT_ELU:
                        nc.scalar.activation(mneg[:, :nn], hT_ps[:, :nn], AF.Relu, scale=-1.0)
                        nc.scalar.activation(mneg[:, :nn], mneg[:, :nn], AF.Exp, scale=-1.0)
                        nc.scalar.activation(relu[:, :nn], hT_ps[:, :nn], AF.Relu)
                    else:
                        nc.vector.tensor_scalar_max(relu[:, :nn], hT_ps[:, :nn], 0.0)
                        nc.vector.tensor_scalar_min(mneg[:, :nn], hT_ps[:, :nn], 0.0)
                        nc.scalar.activation(mneg[:, :nn], mneg[:, :nn], AF.Exp)
                    hT = wp.tile([P, P], dt.float32, tag="hTsb")
                    nc.vector.tensor_tensor(out=hT[:, :nn], in0=relu[:, :nn], in1=mneg[:, :nn], op=OP.add)
                    ps1 = psp1.tile([P, f1 + h1], dt.float32, tag="proj1")
                    nc.tensor.matmul(
                        ps1[:nn, :], lhsT=hT[:, :nn], rhs=w1_sb[:], start=True, stop=True
                    )
                    row = wp.tile([P, f1], dt.bfloat16, tag="row1")
                    nc.vector.tensor_tensor(
                        out=row[:nn, :], in0=ps1[:nn, :f1], in1=w1b_sb[:nn, :f1],
                        op=OP.subtract,
                    )
                    nc.vector.tensor_tensor(
                        out=er1_sb[:nn, t, :], in0=ps1[:nn, f1:], in1=w1b_sb[:nn, f1:],
                        op=OP.subtract,
                    )
                    nc.sync.dma_start(out=tab1_sh[t * P:t * P + nn, :f1], in_=row[:nn, :])

                edge_phase(wp, gp, ip0, tab0, h0, d0, er0_sb, sink0, "0")

        # ---------- context 3: AllGather1 + E1 ----------
        with tile.TileContext(nc) as tc:
            with (
                tc.tile_pool(name="work", bufs=WBUFS) as wp,
                tc.tile_pool(name="gbuf", bufs=GBUFS) as gp,
                tc.tile_pool(name="ipool1", bufs=2) as ip1,
            ):
                nc.gpsimd.collective_compute(
                    "AllGather", mybir.AluOpType.bypass,
                    ins=[tab1_sh[:]], outs=[tab1[:]],
                    replica_groups=[list(range(NC))],
                )

                def sink1(t, nn, orw, rec, hh, dd, wp):
                    ov = wp.tile([P, hh * dd], dt.float32, tag="ov")
                    nc.vector.tensor_tensor(
                        out=ov[:].rearrange("p (h d) -> p h d", h=hh),
                        in0=orw[:].rearrange("p (h d) -> p h d", h=hh),
                        in1=rec[:].to_broadcast([P, hh, dd]),
                        op=OP.mult,
                    )
                    nc.sync.dma_start(out=out_d[t * P:t * P + nn, :], in_=ov[:nn, :])

                edge_phase(wp, gp, ip1, tab1, h1, c1, er1_sb, sink1, "1")

    nc.compile()
    return nc


_CACHE = {}


def build_cached(n_in, h0, d0, h1, c1, Kw, coff, tile_col0, nsh, vsh, ntt):
    key = (n_in, h0, d0, h1, c1, nsh, vsh, ntt, ACT_E, ACT_ELU, Kw.tobytes())
    if key not in _CACHE:
        _CACHE[key] = _build_program(
            n_in, h0, d0, h1, c1, Kw, coff, tile_col0, nsh, vsh, ntt
        )
    return _CACHE[key]


def _rot(al):
    """R with col 0 == al (rest orthonormal complement), and its inverse."""
    d = al.shape[0]
    n = np.linalg.norm(al)
    v = al / n
    # Householder that maps e0 -> v
    w = v.copy()
    w[0] -= 1.0
    if np.linalg.norm(w) < 1e-12:
        Q = np.eye(d)
    else:
        w = w / np.linalg.norm(w)
        Q = np.eye(d) - 2.0 * np.outer(w, w)
    R = Q * 1.0
    R[:, 0] = al
    Rinv = np.linalg.inv(R)
    return R.astype(np.float64), Rinv.astype(np.float64)


def make_in_maps(x, W0, al0, ar0, W1, al1, ar1, perm_c, idx_wrapped):
    n_in = x.shape[1]
    h0, d0 = al0.shape
    h1, c1 = al1.shape
    import ml_dtypes

    bf16 = ml_dtypes.bfloat16

    R0 = np.zeros((h0, d0, d0))
    R0inv = np.zeros((h0, d0, d0))
    for h in range(h0):
        R0[h], R0inv[h] = _rot(al0[h].astype(np.float64))
    W0r = np.einsum("ihd,hde->ihe", W0.reshape(n_in, h0, d0).astype(np.float64), R0)
    W0r = W0r.reshape(n_in, h0 * d0)
    wr0 = np.einsum("ihd,hd->ih", W0.reshape(n_in, h0, d0), ar0)
    w0cat = np.concatenate([W0r, wr0], axis=1).astype(bf16)
    # device: hT = Rinv0^T-free form -> lhsT = block-diag Rinv0 with
    # out[f',p] = sum_f lhsT[f, f'] x0T[f, p];  h = x0B @ Rinv0
    rinv0 = np.zeros((h0 * d0, h0 * d0), np.float64)
    for h in range(h0):
        rinv0[h * d0:(h + 1) * d0, h * d0:(h + 1) * d0] = R0inv[h]
    rinv0 = np.ascontiguousarray(rinv0).astype(np.float32)

    R1 = np.zeros((h1, c1, c1))
    R1inv = np.zeros((h1, c1, c1))
    for h in range(h1):
        R1[h], R1inv[h] = _rot(al1[h].astype(np.float64))
    W1r = np.einsum("ihd,hde->ihe", W1.reshape(h0 * d0, h1, c1).astype(np.float64), R1)
    W1r = W1r.reshape(h0 * d0, h1 * c1)
    wr1 = np.einsum("ihd,hd->ih", W1.reshape(h0 * d0, h1, c1), ar1)
    w1cat = np.ascontiguousarray(
        np.concatenate([W1r, wr1], axis=1)
    ).astype(np.float32)
    w1bias = np.ascontiguousarray(
        np.tile(w1cat.sum(axis=0, dtype=np.float64).astype(np.float32), (128, 1))
    )

    maps = [
        {
            "xT": np.ascontiguousarray(x[perm_c[c]].T).astype(bf16),
            "eidx": np.ascontiguousarray(idx_wrapped[c]),
            "w0cat": np.ascontiguousarray(w0cat),
            "w1cat": w1cat,
            "w1bias": w1bias,
            "rinv0": rinv0,
        }
        for c in range(NC)
    ]
    return maps, R1inv


LAST_EXEC_NS = None
LAST_MEAN_EXEC_NS = None


def kernel(x, src, dst, W0, al0, ar0, W1, al1, ar1):
    x = np.asarray(x, np.float32)
    src = np.asarray(src, np.int32)
    dst = np.asarray(dst, np.int32)
    W0 = np.asarray(W0, np.float32)
    al0 = np.asarray(al0, np.float32)
    ar0 = np.asarray(ar0, np.float32)
    W1 = np.asarray(W1, np.float32)
    al1 = np.asarray(al1, np.float32)
    ar1 = np.asarray(ar1, np.float32)

    n_nodes, n_in = x.shape
    h0, d0 = al0.shape
    h1, c1 = al1.shape

    perm_c, Kw, coff, tile_col0, idx_wrapped, nsh, vsh, ntt = _host_shard(
        src, dst, n_nodes
    )
    nc = build_cached(n_in, h0, d0, h1, c1, Kw, coff, tile_col0, nsh, vsh, ntt)
    in_maps, R1inv = make_in_maps(
        x, W0, al0, ar0, W1, al1, ar1, perm_c, idx_wrapped
    )

    from concourse.bass_utils import run_bass_kernel_spmd

    trace = bool(int(os.environ.get("KERNEL_TRACE", "0")))
    res = run_bass_kernel_spmd(nc, in_maps, list(range(NC)), trace=trace)
    global LAST_EXEC_NS, LAST_MEAN_EXEC_NS
    LAST_EXEC_NS = res.exec_time_ns
    LAST_MEAN_EXEC_NS = res.mean_exec_time_ns
    out = np.empty((n_nodes, c1), np.float64)
    for c in range(NC):
        out[perm_c[c]] = res.results[c]["out"].astype(np.float64)
    # un-rotate layer-1 output on host (mean over heads is identity for h1=1)
    out = out @ R1inv[0]
    return np.ascontiguousarray(out.astype(np.float32))


if __name__ == "__main__":
    pass
